# revision 42
# baseline (speedup 1.0000x reference)
"""Cloth physics step + scatter render on 8 Trainium2 NeuronCores.

Strategy (per sharding_hint): shard the 2048-row grid across 8 cores (256 rows
each).  The 4-direction spring stencil gets its 1-row halos from host-side
shard inputs (single step -> no device halo exchange).  The energy sum is one
tiny AllReduce.  The scatter-render frame is computed per-shard (GPSIMD
local_scatter per vertex-row -> one-hot matmul ORs rows with equal image-Y ->
indirect row-scatter to DRAM) and OR-reduced on host.

Math: newpos = pos + diff * scale with diff = pos - prev + force and
scale = energy_n / (energy + 1e-6), a global scalar (the reference's
vel_dir * vel == diff * (vnorm/max(vnorm,1e-12)) * scale and vnorm >> 1e-12
for this data, so the ratio is exactly 1.0f).

diff is assembled on the PE in PSUM:  diff = (vmat + I) @ pos + 4I @ H
+ (-I) @ prev, where vmat is the vertical tridiagonal stencil (4,-16,4) and
H = left + right + g/4 is built element-wise.  This keeps the Vector engine
off the critical path (fp32 DVE ops on [128,4096] cost ~4.4 us each).
"""

import os
import sys

import numpy as np

for _p in ("/opt/trn_rl_repo",):
    if _p not in sys.path and os.path.isdir(_p):
        sys.path.insert(0, _p)

H, W = 2048, 2048
NCORES = 8
RPC = H // NCORES          # rows per core = 256
E = W * 2                  # interleaved (j, c) elements per row = 4096
IMG = 804
GRAVITY = 9.8
STIFF = 4.0
ALPHA = 0.003
DECAY = 0.99997
XSCALE = (IMG - 20) / float(W)    # 0.3828125 (exact fp32)
YSCALE = (IMG - 700) / float(H)   # 0.05078125 (exact fp32)
CH = 512                   # matmul free-dim chunk (one PSUM bank, fp32)
CGRP = 4                   # chunks per weight-major matmul sweep (PSUM banks)

_CACHE = {}


def _build_kernel():
    import concourse.bass as bass
    import concourse.bacc as bacc
    from concourse import mybir
    from concourse import tile as tile_mod
    from concourse import bass_isa
    from contextlib import ExitStack

    f32 = mybir.dt.float32
    bf16 = mybir.dt.bfloat16
    i16 = mybir.dt.int16
    i32 = mybir.dt.int32
    u8 = mybir.dt.uint8
    Alu = mybir.AluOpType
    Act = mybir.ActivationFunctionType
    AX = mybir.AxisListType

    nc = bacc.Bacc(num_devices=NCORES)

    pos_in = nc.declare_dram_parameter("pos", [RPC, E], f32, isOutput=False)
    prev_in = nc.declare_dram_parameter("prev", [RPC, E], f32, isOutput=False)
    # aux rows: 0 = raw outer halo row, 1 = prev row at the core's global
    # edge (a = top edge, b = bottom edge); mk = pin masks (1 AT pinned).
    auxa_in = nc.declare_dram_parameter("auxa", [2, E], f32, isOutput=False)
    auxb_in = nc.declare_dram_parameter("auxb", [2, E], f32, isOutput=False)
    mkf_in = nc.declare_dram_parameter("mkf", [2, E], f32, isOutput=False)
    vmat_in = nc.declare_dram_parameter("vmat", [128, 128], f32, isOutput=False)
    el_in = nc.declare_dram_parameter("el", [1, 1], f32, isOutput=False)

    newpos_out = nc.declare_dram_parameter("newpos", [RPC, E], f32, isOutput=True)
    occ_out = nc.declare_dram_parameter("occ", [IMG, IMG], u8, isOutput=True)
    eout = nc.declare_dram_parameter("eout", [1, 1], f32, isOutput=True)

    NT = RPC // 128  # row-tiles per core = 2

    with tile_mod.TileContext(nc) as tc, ExitStack() as ctx:
        consts = ctx.enter_context(tc.tile_pool(name="consts", bufs=1))
        pool_pos = ctx.enter_context(tc.tile_pool(name="pos", bufs=2))
        pool_prev = ctx.enter_context(tc.tile_pool(name="prev", bufs=6))
        pool_H = ctx.enter_context(tc.tile_pool(name="H", bufs=2))
        pool_diff = ctx.enter_context(tc.tile_pool(name="diff", bufs=2))
        pool_p2 = ctx.enter_context(tc.tile_pool(name="p2", bufs=2))
        psum_v = ctx.enter_context(tc.tile_pool(name="psv", bufs=CGRP,
                                                space="PSUM"))
        psum_e = ctx.enter_context(tc.tile_pool(name="pse", bufs=1, space="PSUM"))
        psum_o = ctx.enter_context(tc.tile_pool(name="pso", bufs=1, space="PSUM"))
        dram = ctx.enter_context(tc.tile_pool(name="dram", bufs=1, space="DRAM"))

        # ---- big loads first (sync HWDGE ring) ----
        pos_t = []
        for t in range(NT):
            p = pool_pos.tile([128, E], f32, tag="pos")
            nc.sync.dma_start(out=p, in_=pos_in[128 * t:128 * (t + 1), :])
            pos_t.append(p)

        # ---- constants (scalar-engine HWDGE ring) ----
        auxA = consts.tile([1, E], f32)
        auxB = consts.tile([33, E], f32)
        mkf = consts.tile([33, E], f32)
        nc.scalar.dma_start(out=auxA[0:1, :], in_=auxa_in[1:2, :])
        nc.scalar.dma_start(out=auxB[32:33, :], in_=auxb_in[1:2, :])
        nc.scalar.dma_start(out=mkf[0:1, :], in_=mkf_in[0:1, :])   # maskA @0
        nc.scalar.dma_start(out=mkf[32:33, :], in_=mkf_in[1:2, :])  # maskB @32
        vmat = consts.tile([128, 128], f32)
        nc.scalar.dma_start(out=vmat, in_=vmat_in[:, :])

        ones_bf = consts.tile([128, 2048], bf16)
        nc.vector.memset(ones_bf, 1.0)
        iota_i = consts.tile([128, 128], i32)
        nc.gpsimd.iota(iota_i, pattern=[[1, 128]], base=0, channel_multiplier=0)
        iota_f = consts.tile([128, 128], f32)
        nc.gpsimd.tensor_copy(out=iota_f, in_=iota_i)
        iota_pi = consts.tile([128, 1], i32)
        nc.gpsimd.iota(iota_pi, pattern=[[0, 1]], base=0, channel_multiplier=1)
        iota_pf = consts.tile([128, 1], f32)
        nc.gpsimd.tensor_copy(out=iota_pf, in_=iota_pi)
        # identity-based weight for PSUM accumulation of -prev
        ineg = consts.tile([128, 128], f32)
        nc.gpsimd.tensor_scalar(out=ineg, in0=iota_f, scalar1=iota_pf[:, 0:1],
                                op0=Alu.is_equal, scalar2=-1.0, op1=Alu.mult)

        # packed scalar columns: [128, 32] f32
        scal = consts.tile([128, 32], f32)
        C_ONES = 0
        C_ELBC = 1
        C_ESUM = 2       # 2,3 per-tile; 4 total
        C_EMIN = 6
        C_EN2 = 7
        C_TMP = 8
        C_DEN = 9
        C_REC = 10
        C_SCALE = 11
        C_NEL = 12
        C_YMIN = 13      # 13,14 per-tile floored row-min y
        C_YBMIN = 15
        C_DYB = 16       # 16,17 ; +1 in 18,19
        C_ROWF = 20
        C_PIDX = 21
        C_TMP2 = 22
        nc.vector.memset(scal[:, C_ONES:C_ONES + 1], 1.0)
        nc.vector.tensor_copy(out=scal[:, C_PIDX:C_PIDX + 1], in_=iota_pi)
        el_bc = consts.tile([128, 1], f32)
        nc.scalar.dma_start(out=el_bc, in_=el_in[0:1, 0:1].to_broadcast([128, 1]))

        yb16 = consts.tile([128, 2], i16)

        # ---- phase 1: H = l + r + g/4 per tile ----
        H_t = []
        for t in range(NT):
            p = pos_t[t]
            Ht = pool_H.tile([128, E], f32, tag="H")
            H_t.append(Ht)
            eng = nc.vector if t == 0 else nc.gpsimd
            eng.tensor_tensor(out=Ht[:, 2:E - 2], in0=p[:, 0:E - 4],
                              in1=p[:, 4:E], op=Alu.add)
            eng.tensor_tensor(out=Ht[:, 0:2], in0=p[:, 0:2], in1=p[:, 2:4],
                              op=Alu.add)
            eng.tensor_tensor(out=Ht[:, E - 2:E], in0=p[:, E - 4:E - 2],
                              in1=p[:, E - 2:E], op=Alu.add)
            Hy = Ht[:].rearrange("p (j c) -> p j c", c=2)[:, :, 0]
            nc.scalar.activation(out=Hy, in_=Hy, func=Act.Copy,
                                 bias=float(np.float32(-GRAVITY) / 4.0),
                                 scale=1.0)
            # vertical halo rows, raw, straight from DRAM via SWDGE accum
            if t == 0:
                nc.gpsimd.dma_start(out=Ht[0:1, :], in_=auxa_in[0:1, :],
                                    accum_op=Alu.add)
                nc.gpsimd.dma_start(out=Ht[127:128, :], in_=pos_in[128:129, :],
                                    accum_op=Alu.add)
            else:
                nc.gpsimd.dma_start(out=Ht[0:1, :], in_=pos_in[127:128, :],
                                    accum_op=Alu.add)
                nc.gpsimd.dma_start(out=Ht[127:128, :], in_=auxb_in[0:1, :],
                                    accum_op=Alu.add)

        # ---- diff = (vmat+I)@pos + (-I)@prev on PE, then += 4*H on DVE ----
        diff_t = []
        for t in range(NT):
            p = pos_t[t]
            Ht = H_t[t]
            dt_ = pool_diff.tile([128, E], f32, tag="diff")
            diff_t.append(dt_)
            for g in range(E // (CH * CGRP)):
                sls = [slice(CH * (g * CGRP + k), CH * (g * CGRP + k + 1))
                       for k in range(CGRP)]
                pvs = [psum_v.tile([128, CH], f32, tag="psv", name=f"pv{t}{g}{k}")
                       for k in range(CGRP)]
                for pv, sl in zip(pvs, sls):
                    nc.tensor.matmul(pv[:, :], vmat[:, :], p[:, sl],
                                     start=True, stop=False)
                for pv, sl in zip(pvs, sls):
                    pch = pool_prev.tile([128, CH], f32, tag="prev")
                    nc.sync.dma_start(out=pch,
                                      in_=prev_in[128 * t:128 * (t + 1), sl])
                    nc.tensor.matmul(pv[:, :], ineg[:, :], pch[:, :],
                                     start=False, stop=True)
                for pv, sl in zip(pvs, sls):
                    nc.vector.scalar_tensor_tensor(
                        out=dt_[:, sl], in0=Ht[:, sl], scalar=4.0,
                        in1=pv[:, :], op0=Alu.mult, op1=Alu.add)

        # ---- pinned-vertex fix on global edge rows ----
        # Only the pinned columns (strided AP) are touched:
        # diff += mask * ((pos - prev) - diff), so inner cores (mask==0) are
        # no-ops and the SPMD program stays uniform.  H rows are dead and
        # serve as scratch.  Bottom edge (row 127, not an engine-legal base)
        # stages via tiny strided DMAs and lands with a DMA-accumulate.
        def pinap(row_ap, n):
            v = row_ap.rearrange("p (j c) -> p j c", c=2)
            return v[:, 0:9 * (n - 1) + 1:9, :]

        NPIN_A, NPIN_B = W // 9 + 1, (W // 2) // 9 + 1
        sA = pinap(H_t[0][0:1, :], NPIN_A)
        nc.vector.scalar_tensor_tensor(out=sA, in0=pinap(auxA[0:1, :], NPIN_A),
                                       scalar=-1.0,
                                       in1=pinap(pos_t[0][0:1, :], NPIN_A),
                                       op0=Alu.mult, op1=Alu.add)
        nc.vector.tensor_tensor(out=sA, in0=sA,
                                in1=pinap(diff_t[0][0:1, :], NPIN_A),
                                op=Alu.subtract)
        nc.vector.tensor_tensor(out=sA, in0=sA, in1=pinap(mkf[0:1, :], NPIN_A),
                                op=Alu.mult)
        nc.vector.tensor_tensor(out=pinap(diff_t[0][0:1, :], NPIN_A),
                                in0=pinap(diff_t[0][0:1, :], NPIN_A), in1=sA,
                                op=Alu.add)

        sB = pinap(H_t[1][32:33, :], NPIN_B)
        nc.scalar.dma_start(out=sB, in_=pinap(pos_t[NT - 1][127:128, :], NPIN_B))
        nc.vector.scalar_tensor_tensor(out=sB, in0=pinap(auxB[32:33, :], NPIN_B),
                                       scalar=-1.0, in1=sB,
                                       op0=Alu.mult, op1=Alu.add)
        nc.scalar.dma_start(out=pinap(auxB[32:33, :], NPIN_B),
                            in_=pinap(diff_t[NT - 1][127:128, :], NPIN_B))
        nc.vector.tensor_tensor(out=sB, in0=sB,
                                in1=pinap(auxB[32:33, :], NPIN_B),
                                op=Alu.subtract)
        nc.vector.tensor_tensor(out=sB, in0=sB, in1=pinap(mkf[32:33, :], NPIN_B),
                                op=Alu.mult)
        nc.vector.tensor_tensor(out=sB, in0=sB,
                                in1=pinap(auxB[32:33, :], NPIN_B), op=Alu.add)
        nc.scalar.dma_start(out=pinap(diff_t[NT - 1][127:128, :], NPIN_B),
                            in_=sB)

        # ---- local energy: sum(diff^2) via ScalarE Square + accumulate ----
        for t in range(NT):
            nc.scalar.activation(out=H_t[t][:, :], in_=diff_t[t][:, :],
                                 func=Act.Square,
                                 accum_out=scal[:, C_ESUM + t:C_ESUM + t + 1])
        nc.vector.tensor_tensor(out=scal[:, C_ESUM + 2:C_ESUM + 3],
                                in0=scal[:, C_ESUM:C_ESUM + 1],
                                in1=scal[:, C_ESUM + 1:C_ESUM + 2], op=Alu.add)
        pe = psum_e.tile([1, 1], f32)
        nc.tensor.matmul(pe[:, :], scal[:, C_ONES:C_ONES + 1],
                         scal[:, C_ESUM + 2:C_ESUM + 3], start=True, stop=True)
        e_loc = consts.tile([1, 1], f32)
        nc.vector.tensor_copy(out=e_loc, in_=pe[:, :])

        # ---- energy AllReduce across the 8 cores ----
        cc_in = dram.tile([1, 8], f32)
        cc_out = dram.tile([1, 8], f32)
        nc.sync.dma_start(out=cc_in[0:1, 0:1], in_=e_loc[:, :])
        nc.gpsimd.collective_compute(
            "AllReduce", mybir.AluOpType.add,
            replica_groups=[list(range(NCORES))],
            ins=[cc_in[0:1, 0:1]], outs=[cc_out[0:1, 0:1]],
        )
        ebc = consts.tile([128, 1], f32)
        nc.sync.dma_start(out=ebc, in_=cc_out[0:1, 0:1].to_broadcast([128, 1]))

        # ---- global scalars (replicated across partitions) ----
        def sc(c):
            return scal[:, c:c + 1]

        nc.vector.tensor_tensor(out=sc(C_EMIN), in0=ebc, in1=el_bc, op=Alu.min)
        nc.vector.tensor_scalar(out=sc(C_EN2), in0=sc(C_EMIN),
                                scalar1=float(DECAY), scalar2=0.8,
                                op0=Alu.mult, op1=Alu.mult)
        nc.vector.tensor_scalar(out=sc(C_TMP), in0=ebc, scalar1=0.2,
                                op0=Alu.mult, scalar2=None)
        nc.vector.tensor_tensor(out=sc(C_EN2), in0=sc(C_EN2), in1=sc(C_TMP),
                                op=Alu.add)
        nc.vector.tensor_scalar(out=sc(C_DEN), in0=ebc, scalar1=1e-6,
                                op0=Alu.add, scalar2=None)
        nc.vector.reciprocal(out=sc(C_REC), in_=sc(C_DEN))
        nc.vector.tensor_tensor(out=sc(C_SCALE), in0=sc(C_EN2), in1=sc(C_REC),
                                op=Alu.mult)
        nc.vector.tensor_scalar(out=sc(C_NEL), in0=el_bc,
                                scalar1=float(1.0 - ALPHA), op0=Alu.mult,
                                scalar2=None)
        nc.vector.tensor_scalar(out=sc(C_TMP), in0=sc(C_EN2),
                                scalar1=float(ALPHA), op0=Alu.mult, scalar2=None)
        nc.vector.tensor_tensor(out=sc(C_NEL), in0=sc(C_NEL), in1=sc(C_TMP),
                                op=Alu.add)
        nc.scalar.dma_start(out=eout[0:1, 0:1], in_=scal[0:1, C_NEL:C_NEL + 1])

        # ---- phase 2: newpos, image coords, per-row scatter ----
        dst_t = []
        for t in range(NT):
            p = pos_t[t]
            d = diff_t[t]
            # newpos = pos + diff*scale, in place over pos
            nc.vector.scalar_tensor_tensor(out=p[:, :], in0=d[:, :],
                                           scalar=sc(C_SCALE)[:, 0:1],
                                           in1=p[:, :], op0=Alu.mult,
                                           op1=Alu.add)
            nc.scalar.dma_start(out=newpos_out[128 * t:128 * (t + 1), :],
                                in_=p[:, :])

            npv = p[:].rearrange("p (j c) -> p j c", c=2)
            xt = pool_p2.tile([128, 2048], f32, tag="x", bufs=2)
            yt = pool_p2.tile([128, 2048], f32, tag="y", bufs=1)
            # bias shifted by -0.5: the round-to-nearest f32->int cast then
            # computes floor() directly (coords are never exact integers).
            nc.scalar.activation(out=xt, in_=npv[:, :, 1], func=Act.Copy,
                                 scale=XSCALE, bias=10.0 - 0.5)
            nc.scalar.activation(out=yt, in_=npv[:, :, 0], func=Act.Copy,
                                 scale=YSCALE, bias=690.0 - 0.5)
            xi = pool_p2.tile([128, 2048], i16, tag="xi", bufs=1)
            nc.scalar.activation(out=xi, in_=xt, func=Act.Copy)   # floor(x)
            xf2 = pool_p2.tile([128, 2048], f32, tag="xf2", bufs=1)
            # cast back + clamp to the image width in one gpsimd op
            nc.gpsimd.tensor_scalar(out=xf2, in0=xi, scalar1=IMG - 1,
                                    op0=Alu.min, scalar2=None)
            nc.vector.tensor_reduce(out=sc(C_YMIN + t), in_=yt[:, :], axis=AX.X,
                                    op=Alu.min)
            nc.vector.tensor_copy(out=yb16[:, t:t + 1], in_=sc(C_YMIN + t))
            nc.vector.tensor_copy(out=sc(C_YMIN + t), in_=yb16[:, t:t + 1])
            # band = (y' >= ybase + 0.5) * 804 ; key = band + floor(x)
            nc.vector.tensor_scalar(out=sc(C_TMP2), in0=sc(C_YMIN + t),
                                    scalar1=0.5, op0=Alu.add, scalar2=None)
            nc.vector.tensor_scalar(out=yt, in0=yt, scalar1=sc(C_TMP2),
                                    op0=Alu.subtract, scalar2=None)
            nc.vector.tensor_scalar(out=yt, in0=yt, scalar1=0.0,
                                    op0=Alu.is_ge, scalar2=float(IMG),
                                    op1=Alu.mult)
            nc.vector.tensor_tensor(out=xi, in0=yt, in1=xf2, op=Alu.add)
            dst = pool_p2.tile([128, 2 * IMG], bf16, tag="dst")
            dst_t.append(dst)
            nc.gpsimd.local_scatter(dst[:, :], ones_bf[:, :], xi[:, :],
                                    channels=128, num_elems=2 * IMG,
                                    num_idxs=2048)

        # ---- core-level Y window + one-hot OR-combine on the PE ----
        nc.vector.tensor_tensor(out=sc(C_YBMIN), in0=sc(C_YMIN),
                                in1=sc(C_YMIN + 1), op=Alu.min)
        nc.vector.tensor_scalar(out=sc(C_YBMIN), in0=sc(C_YBMIN), scalar1=-1.0,
                                op0=Alu.mult, scalar2=None)
        nc.gpsimd.partition_all_reduce(sc(C_YBMIN), sc(C_YBMIN), channels=128,
                                       reduce_op=bass_isa.ReduceOp.max)
        nc.vector.tensor_scalar(out=sc(C_YBMIN), in0=sc(C_YBMIN), scalar1=-1.0,
                                op0=Alu.mult, scalar2=None)

        onehots = []
        for t in range(NT):
            nc.vector.scalar_tensor_tensor(out=sc(C_DYB + t), in0=sc(C_YBMIN),
                                           scalar=-1.0, in1=sc(C_YMIN + t),
                                           op0=Alu.mult, op1=Alu.add)
            nc.vector.tensor_scalar(out=sc(C_DYB + 2 + t), in0=sc(C_DYB + t),
                                    scalar1=1.0, op0=Alu.add, scalar2=None)
            oh0 = pool_p2.tile([128, 128], bf16, tag="oh0")
            nc.gpsimd.tensor_scalar(out=oh0, in0=iota_f, scalar1=sc(C_DYB + t),
                                    op0=Alu.is_equal, scalar2=None)
            oh1 = pool_p2.tile([128, 128], bf16, tag="oh1")
            nc.gpsimd.tensor_scalar(out=oh1, in0=iota_f,
                                    scalar1=sc(C_DYB + 2 + t),
                                    op0=Alu.is_equal, scalar2=None)
            onehots.append((oh0, oh1))

        po = psum_o.tile([128, IMG], f32)
        for ci, sl in enumerate((slice(0, CH), slice(CH, IMG))):
            for t in range(NT):
                oh0, oh1 = onehots[t]
                nc.tensor.matmul(po[:, sl], oh0[:, :], dst_t[t][:, sl],
                                 start=(t == 0), stop=False)
                nc.tensor.matmul(po[:, sl], oh1[:, :],
                                 dst_t[t][:, IMG + sl.start:IMG + sl.stop],
                                 start=False, stop=(t == NT - 1))

        occ_u8 = pool_p2.tile([128, IMG], u8, tag="occ")
        nc.vector.tensor_scalar(out=occ_u8, in0=po[:, :], scalar1=0.5,
                                op0=Alu.is_gt, scalar2=None)
        nc.vector.tensor_tensor(out=sc(C_ROWF), in0=sc(C_PIDX), in1=sc(C_YBMIN),
                                op=Alu.add)
        rowi = consts.tile([128, 1], i32)
        nc.vector.tensor_copy(out=rowi, in_=sc(C_ROWF))
        nc.gpsimd.indirect_dma_start(
            out=occ_out[:, :],
            out_offset=bass.IndirectOffsetOnAxis(ap=rowi[:, 0:1], axis=0),
            in_=occ_u8[:, :],
            in_offset=None,
            bounds_check=IMG - 1,
            oob_is_err=False,
        )

    nc.compile()
    return nc


def _host_inputs(pos, prev_pos, energy_l):
    pos2 = np.ascontiguousarray(pos, dtype=np.float32).reshape(H, E)
    prev2 = np.ascontiguousarray(prev_pos, dtype=np.float32).reshape(H, E)

    # vertical stencil + identity: V[m] = 4 p[m-1] - 16 p[m] + 4 p[m+1] + p[m]
    vmat = (np.diag(np.full(128, -4.0 * STIFF + 1.0, np.float32))
            + np.diag(np.full(127, STIFF, np.float32), 1)
            + np.diag(np.full(127, STIFF, np.float32), -1)).astype(np.float32)
    el = np.array([[np.float32(energy_l)]], np.float32)

    in_maps = []
    for c in range(NCORES):
        r0 = c * RPC
        auxa = np.zeros((2, E), np.float32)
        auxb = np.zeros((2, E), np.float32)
        mkf = np.zeros((2, E), np.float32)
        auxa[0] = pos2[max(r0 - 1, 0)]          # raw outer halo (dup at edge)
        auxb[0] = pos2[min(r0 + RPC, H - 1)]
        auxa[1] = prev2[r0]
        auxb[1] = prev2[r0 + RPC - 1]
        if c == 0:
            jj = np.arange(0, W, 9)
            mkf[0, 2 * jj] = 1.0
            mkf[0, 2 * jj + 1] = 1.0
        if c == NCORES - 1:
            jj = np.arange(0, W // 2, 9)
            mkf[1, 2 * jj] = 1.0
            mkf[1, 2 * jj + 1] = 1.0
        in_maps.append({
            "pos": pos2[r0:r0 + RPC].copy(),
            "prev": prev2[r0:r0 + RPC].copy(),
            "auxa": auxa,
            "auxb": auxb,
            "mkf": mkf,
            "vmat": vmat,
            "el": el,
        })
    return in_maps


def _run(pos, prev_pos, energy_l, trace=False, **kw):
    from concourse.bass_utils import run_bass_kernel_spmd

    if "nc" not in _CACHE:
        _CACHE["nc"] = _build_kernel()
    nc = _CACHE["nc"]
    in_maps = _host_inputs(pos, prev_pos, energy_l)
    return run_bass_kernel_spmd(nc, in_maps, list(range(NCORES)), trace=trace,
                                **kw)


def _assemble(results):
    newpos = np.concatenate(
        [r["newpos"].reshape(RPC, W, 2) for r in results], axis=0)
    occ = results[0]["occ"]
    for r in results[1:]:
        occ = np.maximum(occ, r["occ"])
    frame = np.zeros((IMG, IMG, 3), np.uint8)
    frame[:, :, 1] = occ * np.uint8(255)
    frame = frame[2:802, 2:802]
    nel = np.float32(results[0]["eout"][0, 0])
    return frame, newpos, nel


def kernel(pos, prev_pos, energy_l):
    res = _run(pos, prev_pos, energy_l, trace=False)
    return _assemble(res.results)


# revision 44
# speedup vs baseline: 1.3363x; 1.3363x over previous
"""Cloth physics step + scatter render on 8 Trainium2 NeuronCores.

Strategy (per sharding_hint): shard the 2048-row grid across 8 cores (256 rows
each).  The 4-direction spring stencil gets its 1-row halos from host-side
shard inputs (single step -> no device halo exchange).  The energy sum is one
tiny AllReduce.  The scatter-render frame is computed per-shard (GPSIMD
local_scatter per vertex-row -> one-hot matmul ORs rows with equal image-Y ->
indirect row-scatter to DRAM) and OR-reduced on host.

Math: newpos = pos + diff * scale with diff = pos - prev + force and
scale = energy_n / (energy + 1e-6), a global scalar (the reference's
vel_dir * vel == diff * (vnorm/max(vnorm,1e-12)) * scale and vnorm >> 1e-12
for this data, so the ratio is exactly 1.0f).

diff is assembled on the PE in PSUM:  diff = (vmat + I) @ pos + 4I @ H
+ (-I) @ prev, where vmat is the vertical tridiagonal stencil (4,-16,4) and
H = left + right + g/4 is built element-wise.  This keeps the Vector engine
off the critical path (fp32 DVE ops on [128,4096] cost ~4.4 us each).
"""

import os
import sys

import numpy as np

for _p in ("/opt/trn_rl_repo",):
    if _p not in sys.path and os.path.isdir(_p):
        sys.path.insert(0, _p)

H, W = 2048, 2048
NCORES = 8
RPC = H // NCORES          # rows per core = 256
E = W * 2                  # interleaved (j, c) elements per row = 4096
IMG = 804
GRAVITY = 9.8
STIFF = 4.0
ALPHA = 0.003
DECAY = 0.99997
XSCALE = (IMG - 20) / float(W)    # 0.3828125 (exact fp32)
YSCALE = (IMG - 700) / float(H)   # 0.05078125 (exact fp32)
CH = 512                   # matmul free-dim chunk (one PSUM bank, fp32)
CGRP = 4                   # chunks per weight-major matmul sweep (PSUM banks)

_CACHE = {}


def _build_kernel():
    import concourse.bass as bass
    import concourse.bacc as bacc
    from concourse import mybir
    from concourse import tile as tile_mod
    from concourse import bass_isa
    from contextlib import ExitStack

    f32 = mybir.dt.float32
    bf16 = mybir.dt.bfloat16
    i16 = mybir.dt.int16
    i32 = mybir.dt.int32
    u8 = mybir.dt.uint8
    Alu = mybir.AluOpType
    Act = mybir.ActivationFunctionType
    AX = mybir.AxisListType

    nc = bacc.Bacc(num_devices=NCORES)

    pos_in = nc.declare_dram_parameter("pos", [RPC, E], f32, isOutput=False)
    prev_in = nc.declare_dram_parameter("prev", [RPC, E], f32, isOutput=False)
    # aux rows: 0 = raw outer halo row, 1 = prev row at the core's global
    # edge (a = top edge, b = bottom edge); mk = pin masks (1 AT pinned).
    auxa_in = nc.declare_dram_parameter("auxa", [2, E], f32, isOutput=False)
    auxb_in = nc.declare_dram_parameter("auxb", [2, E], f32, isOutput=False)
    mkf_in = nc.declare_dram_parameter("mkf", [2, E], f32, isOutput=False)
    vmat_in = nc.declare_dram_parameter("vmat", [128, 128], f32, isOutput=False)
    el_in = nc.declare_dram_parameter("el", [1, 1], f32, isOutput=False)

    newpos_out = nc.declare_dram_parameter("newpos", [RPC, E], f32, isOutput=True)
    occ_out = nc.declare_dram_parameter("occ", [IMG, IMG], u8, isOutput=True)
    eout = nc.declare_dram_parameter("eout", [1, 1], f32, isOutput=True)

    NT = RPC // 128  # row-tiles per core = 2

    with tile_mod.TileContext(nc) as tc, ExitStack() as ctx:
        consts = ctx.enter_context(tc.tile_pool(name="consts", bufs=1))
        pool_pos = ctx.enter_context(tc.tile_pool(name="pos", bufs=2))
        pool_prev = ctx.enter_context(tc.tile_pool(name="prev", bufs=6))
        pool_H = ctx.enter_context(tc.tile_pool(name="H", bufs=2))
        pool_diff = ctx.enter_context(tc.tile_pool(name="diff", bufs=2))
        pool_p2 = ctx.enter_context(tc.tile_pool(name="p2", bufs=2))
        psum_v = ctx.enter_context(tc.tile_pool(name="psv", bufs=CGRP,
                                                space="PSUM"))
        psum_e = ctx.enter_context(tc.tile_pool(name="pse", bufs=1, space="PSUM"))
        psum_o = ctx.enter_context(tc.tile_pool(name="pso", bufs=1, space="PSUM"))
        dram = ctx.enter_context(tc.tile_pool(name="dram", bufs=1, space="DRAM"))

        # ---- big loads first (sync HWDGE ring) ----
        pos_t = []
        for t in range(NT):
            p = pool_pos.tile([128, E], f32, tag="pos")
            nc.sync.dma_start(out=p, in_=pos_in[128 * t:128 * (t + 1), :])
            pos_t.append(p)

        # ---- constants (scalar-engine HWDGE ring) ----
        auxA = consts.tile([1, E], f32)
        auxB = consts.tile([33, E], f32)
        mkf = consts.tile([33, E], f32)
        nc.scalar.dma_start(out=auxA[0:1, :], in_=auxa_in[1:2, :])
        nc.scalar.dma_start(out=auxB[32:33, :], in_=auxb_in[1:2, :])
        nc.scalar.dma_start(out=mkf[0:1, :], in_=mkf_in[0:1, :])   # maskA @0
        nc.scalar.dma_start(out=mkf[32:33, :], in_=mkf_in[1:2, :])  # maskB @32
        vmat = consts.tile([128, 128], f32)
        nc.scalar.dma_start(out=vmat, in_=vmat_in[:, :])

        ones_bf = consts.tile([128, 2048], bf16)
        nc.vector.memset(ones_bf, 1.0)
        iota_i = consts.tile([128, 128], i32)
        nc.gpsimd.iota(iota_i, pattern=[[1, 128]], base=0, channel_multiplier=0)
        iota_f = consts.tile([128, 128], f32)
        nc.vector.tensor_copy(out=iota_f, in_=iota_i)
        iota_pi = consts.tile([128, 1], i32)
        nc.gpsimd.iota(iota_pi, pattern=[[0, 1]], base=0, channel_multiplier=1)
        iota_pf = consts.tile([128, 1], f32)
        nc.vector.tensor_copy(out=iota_pf, in_=iota_pi)
        # identity-based weight for PSUM accumulation of -prev
        ineg = consts.tile([128, 128], f32)
        nc.vector.tensor_scalar(out=ineg, in0=iota_f, scalar1=iota_pf[:, 0:1],
                                op0=Alu.is_equal, scalar2=-1.0, op1=Alu.mult)

        # packed scalar columns: [128, 32] f32
        scal = consts.tile([128, 32], f32)
        C_ONES = 0
        C_ELBC = 1
        C_ESUM = 2       # 2,3 per-tile; 4 total
        C_EMIN = 6
        C_EN2 = 7
        C_TMP = 8
        C_DEN = 9
        C_REC = 10
        C_SCALE = 11
        C_NEL = 12
        C_YMIN = 13      # 13,14 per-tile floored row-min y
        C_YBMIN = 15
        C_DYB = 16       # 16,17 ; +1 in 18,19
        C_ROWF = 20
        C_PIDX = 21
        C_TMP2 = 22
        nc.vector.memset(scal[:, C_ONES:C_ONES + 1], 1.0)
        nc.vector.tensor_copy(out=scal[:, C_PIDX:C_PIDX + 1], in_=iota_pi)
        el_bc = consts.tile([128, 1], f32)
        nc.scalar.dma_start(out=el_bc, in_=el_in[0:1, 0:1].to_broadcast([128, 1]))

        yb16 = consts.tile([128, 2], i16)

        # ---- phase 1: H = l + r + g/4 per tile ----
        H_t = []
        for t in range(NT):
            p = pos_t[t]
            Ht = pool_H.tile([128, E], f32, tag="H")
            H_t.append(Ht)
            eng = nc.vector if t == 0 else nc.gpsimd
            eng.tensor_tensor(out=Ht[:, 2:E - 2], in0=p[:, 0:E - 4],
                              in1=p[:, 4:E], op=Alu.add)
            eng.tensor_tensor(out=Ht[:, 0:2], in0=p[:, 0:2], in1=p[:, 2:4],
                              op=Alu.add)
            eng.tensor_tensor(out=Ht[:, E - 2:E], in0=p[:, E - 4:E - 2],
                              in1=p[:, E - 2:E], op=Alu.add)
            Hy = Ht[:].rearrange("p (j c) -> p j c", c=2)[:, :, 0]
            nc.scalar.activation(out=Hy, in_=Hy, func=Act.Copy,
                                 bias=float(np.float32(-GRAVITY) / 4.0),
                                 scale=1.0)
            # vertical halo rows, raw, straight from DRAM via SWDGE accum
            if t == 0:
                nc.gpsimd.dma_start(out=Ht[0:1, :], in_=auxa_in[0:1, :],
                                    accum_op=Alu.add)
                nc.gpsimd.dma_start(out=Ht[127:128, :], in_=pos_in[128:129, :],
                                    accum_op=Alu.add)
            else:
                nc.gpsimd.dma_start(out=Ht[0:1, :], in_=pos_in[127:128, :],
                                    accum_op=Alu.add)
                nc.gpsimd.dma_start(out=Ht[127:128, :], in_=auxb_in[0:1, :],
                                    accum_op=Alu.add)

        # ---- diff = (vmat+I)@pos + (-I)@prev on PE, then += 4*H on DVE ----
        diff_t = []
        for t in range(NT):
            p = pos_t[t]
            Ht = H_t[t]
            dt_ = pool_diff.tile([128, E], f32, tag="diff")
            diff_t.append(dt_)
            for g in range(E // (CH * CGRP)):
                sls = [slice(CH * (g * CGRP + k), CH * (g * CGRP + k + 1))
                       for k in range(CGRP)]
                pvs = [psum_v.tile([128, CH], f32, tag="psv", name=f"pv{t}{g}{k}")
                       for k in range(CGRP)]
                for pv, sl in zip(pvs, sls):
                    nc.tensor.matmul(pv[:, :], vmat[:, :], p[:, sl],
                                     start=True, stop=False)
                for pv, sl in zip(pvs, sls):
                    pch = pool_prev.tile([128, CH], f32, tag="prev")
                    nc.sync.dma_start(out=pch,
                                      in_=prev_in[128 * t:128 * (t + 1), sl])
                    nc.tensor.matmul(pv[:, :], ineg[:, :], pch[:, :],
                                     start=False, stop=True)
                for pv, sl in zip(pvs, sls):
                    nc.vector.scalar_tensor_tensor(
                        out=dt_[:, sl], in0=Ht[:, sl], scalar=4.0,
                        in1=pv[:, :], op0=Alu.mult, op1=Alu.add)

        # ---- pinned-vertex fix on global edge rows ----
        # Only the pinned columns (strided AP) are touched:
        # diff += mask * ((pos - prev) - diff), so inner cores (mask==0) are
        # no-ops and the SPMD program stays uniform.  H rows are dead and
        # serve as scratch.  Bottom edge (row 127, not an engine-legal base)
        # stages via tiny strided DMAs and lands with a DMA-accumulate.
        def pinap(row_ap, n):
            v = row_ap.rearrange("p (j c) -> p j c", c=2)
            return v[:, 0:9 * (n - 1) + 1:9, :]

        NPIN_A, NPIN_B = W // 9 + 1, (W // 2) // 9 + 1
        sA = pinap(H_t[0][0:1, :], NPIN_A)
        nc.vector.scalar_tensor_tensor(out=sA, in0=pinap(auxA[0:1, :], NPIN_A),
                                       scalar=-1.0,
                                       in1=pinap(pos_t[0][0:1, :], NPIN_A),
                                       op0=Alu.mult, op1=Alu.add)
        nc.vector.tensor_tensor(out=sA, in0=sA,
                                in1=pinap(diff_t[0][0:1, :], NPIN_A),
                                op=Alu.subtract)
        nc.vector.tensor_tensor(out=sA, in0=sA, in1=pinap(mkf[0:1, :], NPIN_A),
                                op=Alu.mult)
        nc.vector.tensor_tensor(out=pinap(diff_t[0][0:1, :], NPIN_A),
                                in0=pinap(diff_t[0][0:1, :], NPIN_A), in1=sA,
                                op=Alu.add)

        sB = pinap(H_t[1][32:33, :], NPIN_B)
        nc.scalar.dma_start(out=sB, in_=pinap(pos_t[NT - 1][127:128, :], NPIN_B))
        nc.vector.scalar_tensor_tensor(out=sB, in0=pinap(auxB[32:33, :], NPIN_B),
                                       scalar=-1.0, in1=sB,
                                       op0=Alu.mult, op1=Alu.add)
        nc.scalar.dma_start(out=pinap(auxB[32:33, :], NPIN_B),
                            in_=pinap(diff_t[NT - 1][127:128, :], NPIN_B))
        nc.vector.tensor_tensor(out=sB, in0=sB,
                                in1=pinap(auxB[32:33, :], NPIN_B),
                                op=Alu.subtract)
        nc.vector.tensor_tensor(out=sB, in0=sB, in1=pinap(mkf[32:33, :], NPIN_B),
                                op=Alu.mult)
        nc.vector.tensor_tensor(out=sB, in0=sB,
                                in1=pinap(auxB[32:33, :], NPIN_B), op=Alu.add)
        nc.scalar.dma_start(out=pinap(diff_t[NT - 1][127:128, :], NPIN_B),
                            in_=sB)

        # ---- local energy: sum(diff^2) via ScalarE Square + accumulate ----
        for t in range(NT):
            nc.scalar.activation(out=H_t[t][:, :], in_=diff_t[t][:, :],
                                 func=Act.Square,
                                 accum_out=scal[:, C_ESUM + t:C_ESUM + t + 1])
        nc.vector.tensor_tensor(out=scal[:, C_ESUM + 2:C_ESUM + 3],
                                in0=scal[:, C_ESUM:C_ESUM + 1],
                                in1=scal[:, C_ESUM + 1:C_ESUM + 2], op=Alu.add)
        pe = psum_e.tile([1, 1], f32)
        nc.tensor.matmul(pe[:, :], scal[:, C_ONES:C_ONES + 1],
                         scal[:, C_ESUM + 2:C_ESUM + 3], start=True, stop=True)
        e_loc = consts.tile([1, 1], f32)
        nc.vector.tensor_copy(out=e_loc, in_=pe[:, :])

        # ---- energy AllReduce across the 8 cores ----
        cc_in = dram.tile([1, 8], f32)
        cc_out = dram.tile([1, 8], f32)
        nc.sync.dma_start(out=cc_in[0:1, 0:1], in_=e_loc[:, :])
        nc.gpsimd.collective_compute(
            "AllReduce", mybir.AluOpType.add,
            replica_groups=[list(range(NCORES))],
            ins=[cc_in[0:1, 0:1]], outs=[cc_out[0:1, 0:1]],
        )
        ebc = consts.tile([128, 1], f32)
        nc.sync.dma_start(out=ebc, in_=cc_out[0:1, 0:1].to_broadcast([128, 1]))

        # ---- global scalars (replicated across partitions) ----
        def sc(c):
            return scal[:, c:c + 1]

        nc.vector.tensor_tensor(out=sc(C_EMIN), in0=ebc, in1=el_bc, op=Alu.min)
        nc.vector.tensor_scalar(out=sc(C_EN2), in0=sc(C_EMIN),
                                scalar1=float(DECAY), scalar2=0.8,
                                op0=Alu.mult, op1=Alu.mult)
        nc.vector.tensor_scalar(out=sc(C_TMP), in0=ebc, scalar1=0.2,
                                op0=Alu.mult, scalar2=None)
        nc.vector.tensor_tensor(out=sc(C_EN2), in0=sc(C_EN2), in1=sc(C_TMP),
                                op=Alu.add)
        nc.vector.tensor_scalar(out=sc(C_DEN), in0=ebc, scalar1=1e-6,
                                op0=Alu.add, scalar2=None)
        nc.vector.reciprocal(out=sc(C_REC), in_=sc(C_DEN))
        nc.vector.tensor_tensor(out=sc(C_SCALE), in0=sc(C_EN2), in1=sc(C_REC),
                                op=Alu.mult)
        nc.vector.tensor_scalar(out=sc(C_NEL), in0=el_bc,
                                scalar1=float(1.0 - ALPHA), op0=Alu.mult,
                                scalar2=None)
        nc.vector.tensor_scalar(out=sc(C_TMP), in0=sc(C_EN2),
                                scalar1=float(ALPHA), op0=Alu.mult, scalar2=None)
        nc.vector.tensor_tensor(out=sc(C_NEL), in0=sc(C_NEL), in1=sc(C_TMP),
                                op=Alu.add)
        nc.scalar.dma_start(out=eout[0:1, 0:1], in_=scal[0:1, C_NEL:C_NEL + 1])

        # ---- phase 2: newpos, image coords, per-row scatter ----
        dst_t = []
        for t in range(NT):
            p = pos_t[t]
            d = diff_t[t]
            # newpos = pos + diff*scale, in place over pos
            nc.vector.scalar_tensor_tensor(out=p[:, :], in0=d[:, :],
                                           scalar=sc(C_SCALE)[:, 0:1],
                                           in1=p[:, :], op0=Alu.mult,
                                           op1=Alu.add)
            nc.scalar.dma_start(out=newpos_out[128 * t:128 * (t + 1), :],
                                in_=p[:, :])

            npv = p[:].rearrange("p (j c) -> p j c", c=2)
            xt = pool_p2.tile([128, 2048], f32, tag="x", bufs=2)
            yt = pool_p2.tile([128, 2048], f32, tag="y", bufs=1)
            # bias shifted by -0.5: the round-to-nearest f32->int cast then
            # computes floor() directly (coords are never exact integers).
            nc.scalar.activation(out=xt, in_=npv[:, :, 1], func=Act.Copy,
                                 scale=XSCALE, bias=10.0 - 0.5)
            nc.scalar.activation(out=yt, in_=npv[:, :, 0], func=Act.Copy,
                                 scale=YSCALE, bias=690.0 - 0.5)
            xi = pool_p2.tile([128, 2048], i16, tag="xi", bufs=1)
            nc.scalar.activation(out=xi, in_=xt, func=Act.Copy)   # floor(x)
            xf2 = pool_p2.tile([128, 2048], f32, tag="xf2", bufs=1)
            # cast back + clamp to the image width in one op
            nc.vector.tensor_scalar(out=xf2, in0=xi, scalar1=IMG - 1,
                                    op0=Alu.min, scalar2=None)
            nc.vector.tensor_reduce(out=sc(C_YMIN + t), in_=yt[:, :], axis=AX.X,
                                    op=Alu.min)
            nc.vector.tensor_copy(out=yb16[:, t:t + 1], in_=sc(C_YMIN + t))
            nc.vector.tensor_copy(out=sc(C_YMIN + t), in_=yb16[:, t:t + 1])
            # band = (y' >= ybase + 0.5) * 804 ; key = band + floor(x)
            nc.vector.tensor_scalar(out=sc(C_TMP2), in0=sc(C_YMIN + t),
                                    scalar1=-1.0, op0=Alu.mult, scalar2=-0.5,
                                    op1=Alu.add)
            nc.scalar.activation(out=yt, in_=yt, func=Act.Identity,
                                 bias=sc(C_TMP2))
            nc.vector.tensor_scalar(out=yt, in0=yt, scalar1=0.0,
                                    op0=Alu.is_ge, scalar2=float(IMG),
                                    op1=Alu.mult)
            nc.vector.tensor_tensor(out=xi, in0=yt, in1=xf2, op=Alu.add)
            dst = pool_p2.tile([128, 2 * IMG], bf16, tag="dst")
            dst_t.append(dst)
            nc.gpsimd.local_scatter(dst[:, :], ones_bf[:, :], xi[:, :],
                                    channels=128, num_elems=2 * IMG,
                                    num_idxs=2048)

        # ---- core-level Y window + one-hot OR-combine on the PE ----
        nc.vector.tensor_tensor(out=sc(C_YBMIN), in0=sc(C_YMIN),
                                in1=sc(C_YMIN + 1), op=Alu.min)
        nc.vector.tensor_scalar(out=sc(C_YBMIN), in0=sc(C_YBMIN), scalar1=-1.0,
                                op0=Alu.mult, scalar2=None)
        nc.gpsimd.partition_all_reduce(sc(C_YBMIN), sc(C_YBMIN), channels=128,
                                       reduce_op=bass_isa.ReduceOp.max)
        nc.vector.tensor_scalar(out=sc(C_YBMIN), in0=sc(C_YBMIN), scalar1=-1.0,
                                op0=Alu.mult, scalar2=None)

        onehots = []
        for t in range(NT):
            nc.vector.scalar_tensor_tensor(out=sc(C_DYB + t), in0=sc(C_YBMIN),
                                           scalar=-1.0, in1=sc(C_YMIN + t),
                                           op0=Alu.mult, op1=Alu.add)
            nc.vector.tensor_scalar(out=sc(C_DYB + 2 + t), in0=sc(C_DYB + t),
                                    scalar1=1.0, op0=Alu.add, scalar2=None)
            oh0 = pool_p2.tile([128, 128], bf16, tag="oh0")
            nc.vector.tensor_scalar(out=oh0, in0=iota_f, scalar1=sc(C_DYB + t),
                                    op0=Alu.is_equal, scalar2=None)
            oh1 = pool_p2.tile([128, 128], bf16, tag="oh1")
            nc.vector.tensor_scalar(out=oh1, in0=iota_f,
                                    scalar1=sc(C_DYB + 2 + t),
                                    op0=Alu.is_equal, scalar2=None)
            onehots.append((oh0, oh1))

        po = psum_o.tile([128, IMG], f32)
        for ci, sl in enumerate((slice(0, CH), slice(CH, IMG))):
            for t in range(NT):
                oh0, oh1 = onehots[t]
                nc.tensor.matmul(po[:, sl], oh0[:, :], dst_t[t][:, sl],
                                 start=(t == 0), stop=False)
                nc.tensor.matmul(po[:, sl], oh1[:, :],
                                 dst_t[t][:, IMG + sl.start:IMG + sl.stop],
                                 start=False, stop=(t == NT - 1))

        occ_u8 = pool_p2.tile([128, IMG], u8, tag="occ")
        nc.vector.tensor_scalar(out=occ_u8, in0=po[:, :], scalar1=0.5,
                                op0=Alu.is_gt, scalar2=None)
        nc.vector.tensor_tensor(out=sc(C_ROWF), in0=sc(C_PIDX), in1=sc(C_YBMIN),
                                op=Alu.add)
        rowi = consts.tile([128, 1], i32)
        nc.vector.tensor_copy(out=rowi, in_=sc(C_ROWF))
        nc.gpsimd.indirect_dma_start(
            out=occ_out[:, :],
            out_offset=bass.IndirectOffsetOnAxis(ap=rowi[:, 0:1], axis=0),
            in_=occ_u8[:, :],
            in_offset=None,
            bounds_check=IMG - 1,
            oob_is_err=False,
        )

    nc.compile()
    return nc


def _host_inputs(pos, prev_pos, energy_l):
    pos2 = np.ascontiguousarray(pos, dtype=np.float32).reshape(H, E)
    prev2 = np.ascontiguousarray(prev_pos, dtype=np.float32).reshape(H, E)

    # vertical stencil + identity: V[m] = 4 p[m-1] - 16 p[m] + 4 p[m+1] + p[m]
    vmat = (np.diag(np.full(128, -4.0 * STIFF + 1.0, np.float32))
            + np.diag(np.full(127, STIFF, np.float32), 1)
            + np.diag(np.full(127, STIFF, np.float32), -1)).astype(np.float32)
    el = np.array([[np.float32(energy_l)]], np.float32)

    in_maps = []
    for c in range(NCORES):
        r0 = c * RPC
        auxa = np.zeros((2, E), np.float32)
        auxb = np.zeros((2, E), np.float32)
        mkf = np.zeros((2, E), np.float32)
        auxa[0] = pos2[max(r0 - 1, 0)]          # raw outer halo (dup at edge)
        auxb[0] = pos2[min(r0 + RPC, H - 1)]
        auxa[1] = prev2[r0]
        auxb[1] = prev2[r0 + RPC - 1]
        if c == 0:
            jj = np.arange(0, W, 9)
            mkf[0, 2 * jj] = 1.0
            mkf[0, 2 * jj + 1] = 1.0
        if c == NCORES - 1:
            jj = np.arange(0, W // 2, 9)
            mkf[1, 2 * jj] = 1.0
            mkf[1, 2 * jj + 1] = 1.0
        in_maps.append({
            "pos": pos2[r0:r0 + RPC].copy(),
            "prev": prev2[r0:r0 + RPC].copy(),
            "auxa": auxa,
            "auxb": auxb,
            "mkf": mkf,
            "vmat": vmat,
            "el": el,
        })
    return in_maps


def _run(pos, prev_pos, energy_l, trace=False, **kw):
    from concourse.bass_utils import run_bass_kernel_spmd

    if "nc" not in _CACHE:
        _CACHE["nc"] = _build_kernel()
    nc = _CACHE["nc"]
    in_maps = _host_inputs(pos, prev_pos, energy_l)
    return run_bass_kernel_spmd(nc, in_maps, list(range(NCORES)), trace=trace,
                                **kw)


def _assemble(results):
    newpos = np.concatenate(
        [r["newpos"].reshape(RPC, W, 2) for r in results], axis=0)
    occ = results[0]["occ"]
    for r in results[1:]:
        occ = np.maximum(occ, r["occ"])
    frame = np.zeros((IMG, IMG, 3), np.uint8)
    frame[:, :, 1] = occ * np.uint8(255)
    frame = frame[2:802, 2:802]
    nel = np.float32(results[0]["eout"][0, 0])
    return frame, newpos, nel


def kernel(pos, prev_pos, energy_l):
    res = _run(pos, prev_pos, energy_l, trace=False)
    return _assemble(res.results)
